# revision 28
# baseline (speedup 1.0000x reference)
"""AWBNet (wo R2) Trainium2 kernel — v7.

Math (per sample b):
  m = reshape(relu(hist_flat @ W1 + b1) @ W2 + b2, [9, 3])
  feats(px) = [r, g, b, r^2, g^2, b^2, rg, rb, gb]
  y[px, c] = sum_k feats[px, k] * m[k, c]

Device strategy (8 cores, data parallel over batch, 2 samples/core).
Cross-core collectives measured ~60-90us on this backend, so W1 stays
replicated; the schedule makes its 6.3MB stream the *only* thing ahead
of the per-pixel storm:

  * W1 is chunk-streamed on both HWDGE rings (small leading chunks so
    the first matmuls start early); the MLP matmuls chase chunk
    arrival in k-order. b1 rides as one extra k-tile (hp row of ones),
    b2 as a rank-3 matmul into the same PSUM accumulation group, so
    the post-W1 critical chain is:
      relu (ACT, from PSUM) -> 6 patt MMs -> patt evict (ACT, split)
      -> 7 band MMs -> msP9 evict (ACT) -> 9 masked tensor_scalar
      builds (DVE) -> storm.
    The whole chain runs under tc.high_priority() so the Tile
    scheduler cannot queue storm sq/cr ops ahead of it on the
    in-order DVE/ACT engines.
  * xi / xr(=rotated xi rows, read straight from xi_d) stream behind
    W1, stage-interleaved across both rings so arrivals track storm
    consumption; xr stages 4-7 are SBUF->SBUF copies on gpsimd.
  * storm: 8 column stages, 3 block-diagonal matmuls per 512-col
    chunk, PSUM ring of 2, y evict on ACT, y DMA alternating rings.
"""

import sys

import numpy as np

for _p in ("/opt/trn_rl_repo",):
    if _p not in sys.path:
        sys.path.insert(0, _p)

import concourse.bacc as bacc
import concourse.mybir as mybir
import concourse.tile as tile
from concourse import bass_utils

# ---- problem constants (hardcoded per contract) ----
N_CORES = 8
B, H, W, C = 16, 512, 512, 3
SPC = B // N_CORES  # samples per core = 2
PX_SAMPLE = H * W  # 262144
P = 128

G_S = 21  # pixel groups per sample
G = SPC * G_S  # 42 groups
NP = 3 * G  # 126 used partitions
XCOLS = 12544  # padded pixels per group (21*12544 >= 262144)

STAGE_COLS = (512, 1024, 2048, 2048, 2048, 2048, 2048, 768)
NSTAGE = len(STAGE_COLS)
XR_HBM_STAGES = 5  # stages 0-4 xr from HBM; 5-7 SBUF->SBUF

HIST = 3 * 64 * 64  # 12288
HID = 256
MOUT = 27
KT = HIST // P  # 96 k-tiles
KTB = KT + 1  # +1 bias k-tile (hp row of ones x b1)
MT = HID // P  # 2 m-tiles
# W1 chunk sizes (k-tiles): small first chunk (incl bias tile) -> early
# MM start; small last chunks -> short chase lag; alternate sync/scalar.
W1_CHUNKS = (4, 8, 8, 8, 8, 8, 8, 8, 8, 8, 8, 5, 4, 4)
assert sum(W1_CHUNKS) == KTB

F16 = mybir.dt.float16
F32 = mybir.dt.float32
MULT = mybir.AluOpType.mult
RELU = mybir.ActivationFunctionType.Relu

_CACHE = {}


def _colmap(mat, i, c):
    """W2/b2 column for (matrix, band, out-channel): which of the 27
    m-coefficients scales plane_mat band i into channel c."""
    if mat == 0:  # linear: x_i
        k = i
    elif mat == 1:  # squares: x_i^2
        k = 3 + i
    else:  # crosses: x_i * x_{(i+1)%3} -> rg, gb, br
        k = (6, 8, 7)[i]
    return 3 * k + c


def _build():
    nc = bacc.Bacc(
        "TRN2", target_bir_lowering=False, debug=False, num_devices=N_CORES
    )

    xi_d = nc.dram_tensor("xi", [NP, XCOLS], F16, kind="ExternalInput")
    w1_d = nc.dram_tensor("w1pm", [P, KTB, HID], F16, kind="ExternalInput")
    hp_d = nc.dram_tensor("h_packed", [P, KTB, SPC], F16, kind="ExternalInput")
    w2_d = nc.dram_tensor("w2i", [MT, P, 3 * 9], F16, kind="ExternalInput")
    bsel_d = nc.dram_tensor("bsel3", [3, NP], F16, kind="ExternalInput")
    b2r_d = nc.dram_tensor("b2r3", [3, 9], F16, kind="ExternalInput")
    e3_d = nc.dram_tensor("e3", [SPC, 3, NP], F16, kind="ExternalInput")
    mask_d = nc.dram_tensor("maskS", [NP, G], F16, kind="ExternalInput")
    y_d = nc.dram_tensor("y_bands", [NP, XCOLS], F16, kind="ExternalOutput")

    offs = [0]
    for ncols in STAGE_COLS:
        offs.append(offs[-1] + ncols)
    assert offs[-1] == XCOLS

    with tile.TileContext(nc) as tc:
        with (
            tc.tile_pool(name="mlp", bufs=1) as mlp_pool,
            tc.tile_pool(name="w1s", bufs=1) as w1_pool,
            tc.tile_pool(name="px", bufs=1) as px_pool,
            tc.tile_pool(name="sqcr", bufs=3) as sqcr_pool,
            tc.tile_pool(name="yring", bufs=4) as y_pool,
        ):
            # ---------------- input DMAs ----------------
            # hp first on sync (needed by the first MM), then W1 chunks
            # alternating sync/scalar in k-order.
            hp_sb = mlp_pool.tile([P, KTB, SPC], F16, tag="hp", name="hp")
            nc.sync.dma_start(out=hp_sb, in_=hp_d[:, :, :])

            # small setup tensors first on gpsimd (fast, tiny)
            w2_sb = mlp_pool.tile([P, MT, 3 * 9], F16, tag="w2", name="w2")
            nc.gpsimd.dma_start(out=w2_sb, in_=w2_d.rearrange("m p n -> p m n"))
            bsel_sb = mlp_pool.tile([3, NP], F16, tag="bsel", name="bsel")
            nc.gpsimd.dma_start(out=bsel_sb, in_=bsel_d[:, :])
            b2r_sb = mlp_pool.tile([3, 9], F16, tag="b2r", name="b2r")
            nc.gpsimd.dma_start(out=b2r_sb, in_=b2r_d[:, :])
            e3_sb = mlp_pool.tile([SPC, 3, NP], F16, tag="e3", name="e3")
            nc.gpsimd.dma_start(out=e3_sb, in_=e3_d[:, :, :])
            mask_sb = mlp_pool.tile([NP, 1, G], F16, tag="mask", name="mask")
            nc.gpsimd.dma_start(out=mask_sb[:, 0, :], in_=mask_d[:, :])

            # W1 chunks alternate the two HWDGE rings in k-order (a third
            # SWDGE stream measurably REDUCES aggregate HBM throughput).
            # The LAST chunk pair is issued after xi_0/xr_0 so stage-0
            # basis inputs land just before the MLP finishes: even if the
            # scheduler emits sq_0/cr_0 ahead of the m-tail on the
            # in-order DVE queue, they can only block it briefly.
            xi_sb = px_pool.tile([NP, XCOLS], F16, tag="xi", name="xi")
            xr_sb = px_pool.tile([NP, XCOLS], F16, tag="xr", name="xr")

            def xi_dma(q, st):
                sl = slice(offs[st], offs[st + 1])
                q.dma_start(out=xi_sb[:, sl], in_=xi_d[:, sl])

            def xr_hbm_dma(q, st):
                sl = slice(offs[st], offs[st + 1])
                q.dma_start(out=xr_sb[0:84, sl], in_=xi_d[42:126, sl])
                q.dma_start(out=xr_sb[84:126, sl], in_=xi_d[0:42, sl])

            w1_sbs = [None] * len(W1_CHUNKS)
            w1_offs = []
            k0 = 0
            for csz in W1_CHUNKS:
                w1_offs.append(k0)
                k0 += csz

            def w1_dma(kc):
                csz = W1_CHUNKS[kc]
                w1_sb = w1_pool.tile(
                    [P, csz, HID], F16, tag=f"w1c{kc}", name=f"w1c{kc}"
                )
                q = nc.sync if kc % 2 == 0 else nc.scalar
                q.dma_start(
                    out=w1_sb, in_=w1_d[:, w1_offs[kc] : w1_offs[kc] + csz, :]
                )
                w1_sbs[kc] = (w1_offs[kc], csz, w1_sb)

            for kc in range(len(W1_CHUNKS) - 2):
                w1_dma(kc)
            xi_dma(nc.sync, 0)
            xr_hbm_dma(nc.scalar, 0)
            w1_dma(len(W1_CHUNKS) - 2)
            w1_dma(len(W1_CHUNKS) - 1)

            xi_dma(nc.scalar, 1)
            xr_hbm_dma(nc.sync, 1)
            xi_dma(nc.sync, 2)
            xr_hbm_dma(nc.scalar, 2)
            xi_dma(nc.scalar, 3)
            xr_hbm_dma(nc.sync, 3)
            xi_dma(nc.sync, 4)
            xr_hbm_dma(nc.scalar, 4)
            xi_dma(nc.scalar, 5)
            xi_dma(nc.sync, 6)
            xi_dma(nc.scalar, 7)

            # xr stages 5-7 SBUF->SBUF on gpsimd. The m-tail compute ops
            # also live on gpsimd, so push these DMAs' scheduler-sim
            # readiness far out to guarantee they are emitted AFTER the
            # m-tail in the in-order gpsimd queue (no runtime effect —
            # real start is gated by the xi chunk semaphores).
            for st in range(XR_HBM_STAGES, NSTAGE):
                sl = slice(offs[st], offs[st + 1])
                with tc.tile_wait_until(0.2 + 0.01 * st):
                    nc.gpsimd.dma_start(
                        out=xr_sb[0:84, sl], in_=xi_sb[42:126, sl]
                    )
                    nc.gpsimd.dma_start(
                        out=xr_sb[84:126, sl], in_=xi_sb[0:42, sl]
                    )

            # ---------------- MLP (TensorE) ----------------
            with tc.tile_pool(name="mlpps", bufs=1, space="PSUM") as mlp_psum:
                feat_ps = mlp_psum.tile([SPC, HID], F32, tag="featps", name="featps")
                for kc, (k0, csz, w1_sb) in enumerate(w1_sbs):
                    for kk in range(csz):
                        k = k0 + kk
                        nc.tensor.matmul(
                            feat_ps,
                            hp_sb[:, k, :],
                            w1_sb[:, kk, :],
                            start=(k == 0),
                            stop=(k == KTB - 1),
                        )

                pt_ps = mlp_psum.tile([P, 2 * 3, P], F32, tag="ptps", name="ptps")
                msP_ps = mlp_psum.tile([NP, 9, 1], F32, tag="msps", name="msps")
                patt = mlp_pool.tile([P, 2 * 3, P], F16, tag="patt", name="patt")
                feat_r = mlp_pool.tile([SPC, HID], F16, tag="featr", name="featr")
                # lhsT viewed [NP, 9, G]: (mat, c) block j=3*mat+c, so
                # lhsT[:, 3*mat:3*mat+3, :] is the [126, 126] stationary
                # for plane `mat` (identical memory layout to [NP, 3, NP]).
                lhsT = mlp_pool.tile([NP, 3 * 3, G], F16, tag="lhsT", name="lhsT")

                # m-tail: PSUM-reading ops must be on DVE (GPSIMD has no
                # PSUM access; ACT is serializing DMA issues here); the
                # final SBUF-only build runs on gpsimd.
                nc.vector.tensor_scalar(
                    feat_r, feat_ps, 0.0, None, mybir.AluOpType.max
                )

                # patt[h, 3*mt+i, o] = feat[s(o), mt*128+h] masked to
                # band i of sample s(o)
                for mt in range(MT):
                    for i in range(3):
                        nc.tensor.matmul(
                            pt_ps[:, 3 * mt + i, 0:NP],
                            feat_r[:, mt * P : (mt + 1) * P],
                            e3_sb[:, i, :],
                            start=True,
                            stop=True,
                        )
                # split evict so band MMs for mt=0 can start early
                nc.vector.tensor_copy(patt[:, 0:3, :], pt_ps[:, 0:3, :])
                nc.vector.tensor_copy(patt[:, 3:6, :], pt_ps[:, 3:6, :])

                # msP[o, j] = b2i[o, j] + sum_h patt[h, o] * w2[...]
                nc.tensor.matmul(
                    msP_ps[:, :, 0], bsel_sb, b2r_sb, start=True, stop=False
                )
                nmm = 0
                for mt in range(MT):
                    for i in range(3):
                        nc.tensor.matmul(
                            msP_ps[:, :, 0],
                            patt[:, 3 * mt + i, 0:NP],
                            w2_sb[:, mt, 9 * i : 9 * (i + 1)],
                            start=False,
                            stop=(nmm == 2 * MT + 1),
                        )
                        nmm += 1

                # evict msP to SBUF (DVE, tiny), then build the
                # block-diagonal stationaries in ONE broadcast op on
                # gpsimd: lhsT[p, j, g] = mask[p, g] * msP9[p, j]
                msP9 = mlp_pool.tile([NP, 9, 1], F32, tag="msP9", name="msP9")
                nc.vector.tensor_copy(msP9, msP_ps)
                import concourse.bass as bass_mod

                bc_mask, bc_ms = bass_mod.broadcast_tensor_aps(
                    mask_sb[:, :, :], msP9[:, :, :]
                )
                nc.gpsimd.tensor_tensor(
                    lhsT[:, :, :], bc_mask, bc_ms, MULT
                )

            # ---------------- storm: per-stage basis + matmuls ----------
            with tc.tile_pool(name="pxps", bufs=2, space="PSUM") as px_psum:
                for st in range(NSTAGE):
                    col0 = offs[st]
                    ncols = STAGE_COLS[st]
                    sl = slice(col0, col0 + ncols)

                    sq_t = sqcr_pool.tile([NP, 2048], F16, tag="sq", name=f"sq{st}")
                    nc.vector.tensor_mul(
                        sq_t[:, 0:ncols], xi_sb[:, sl], xi_sb[:, sl]
                    )
                    cr_t = sqcr_pool.tile([NP, 2048], F16, tag="cr", name=f"cr{st}")
                    nc.vector.tensor_mul(
                        cr_t[:, 0:ncols], xi_sb[:, sl], xr_sb[:, sl]
                    )

                    yc_ps = px_psum.tile([NP, 2048], F32, tag="yc", name=f"yc{st}")
                    nch = (ncols + 511) // 512
                    for mat in range(3):
                        for n in range(nch):
                            c0 = n * 512
                            c1 = min(c0 + 512, ncols)
                            if mat == 0:
                                rhs = xi_sb[:, col0 + c0 : col0 + c1]
                            elif mat == 1:
                                rhs = sq_t[:, c0:c1]
                            else:
                                rhs = cr_t[:, c0:c1]
                            nc.tensor.matmul(
                                yc_ps[:, c0:c1],
                                lhsT[:, 3 * mat : 3 * mat + 3, :],
                                rhs,
                                start=(mat == 0),
                                stop=(mat == 2),
                            )

                    y_sb = y_pool.tile([NP, 2048], F16, tag="ysb", name=f"y{st}")
                    nc.scalar.copy(y_sb[:, 0:ncols], yc_ps[:, 0:ncols])
                    yq = nc.sync if st % 2 == 0 else nc.scalar
                    yq.dma_start(out=y_d[:, sl], in_=y_sb[:, 0:ncols])

    nc.compile()
    return nc


def _prep_inputs(x, histogram, W1, b1, W2, b2):
    """Host-side sharding / layout packing (layout + dtype only; no data
    arithmetic)."""
    x = np.asarray(x, dtype=np.float32)
    hist = np.asarray(histogram, dtype=np.float32).reshape(B, HIST)
    W1 = np.asarray(W1, dtype=np.float32)
    b1 = np.asarray(b1, dtype=np.float32)
    W2 = np.asarray(W2, dtype=np.float32)
    b2 = np.asarray(b2, dtype=np.float32)

    # w1pm[p, kt, n]: k-tile 0 is the bias tile (b1 in partition 0)
    w1pm = np.zeros((P, KTB, HID), dtype=np.float16)
    w1pm[:, 1:, :] = W1.reshape(KT, P, HID).transpose(1, 0, 2).astype(np.float16)
    w1pm[0, 0, :] = b1.astype(np.float16)

    e3 = np.zeros((SPC, 3, NP), dtype=np.float16)
    for i in range(3):
        for s in range(SPC):
            e3[s, i, 42 * i + G_S * s : 42 * i + G_S * (s + 1)] = 1.0

    # W2 / b2 with interleave-mapped columns
    cm = np.empty((3, 9), dtype=np.int64)  # [i, 3*mat+c]
    for i in range(3):
        for mat in range(3):
            for c in range(C):
                cm[i, 3 * mat + c] = _colmap(mat, i, c)
    w2i = np.ascontiguousarray(
        W2.reshape(MT, P, MOUT)[:, :, cm.reshape(-1)].reshape(MT, P, 3, 9)
        .reshape(MT, P, 27)
    ).astype(np.float16)
    # rank-3 bias: msP[o, j] += sum_i bsel[i, o] * b2r[i, j]
    bsel = np.zeros((3, NP), dtype=np.float16)
    for i in range(3):
        bsel[i, 42 * i : 42 * (i + 1)] = 1.0
    b2r = np.empty((3, 9), dtype=np.float16)
    for i in range(3):
        b2r[i] = b2[cm[i]].astype(np.float16)

    maskS = np.zeros((NP, G), dtype=np.float16)
    for i in range(3):
        for g in range(G):
            maskS[42 * i + g, g] = 1.0

    in_maps = []
    for core in range(N_CORES):
        xI = np.zeros((NP, XCOLS), dtype=np.float16)
        for s in range(SPC):
            xs = x[core * SPC + s].reshape(PX_SAMPLE, C)
            pad = np.zeros((G_S * XCOLS, C), dtype=np.float32)
            pad[:PX_SAMPLE] = xs
            v = pad.reshape(G_S, XCOLS, C)  # [g', n, i]
            for i in range(3):
                xI[42 * i + G_S * s : 42 * i + G_S * (s + 1), :] = v[:, :, i].astype(
                    np.float16
                )

        h_core = hist[core * SPC : (core + 1) * SPC]
        hp = np.zeros((P, KTB, SPC), dtype=np.float16)
        hp[:, 1:, :] = (
            h_core.reshape(SPC, KT, P).transpose(2, 1, 0)
        ).astype(np.float16)
        hp[0, 0, :] = 1.0  # bias row: 1 * b1
        in_maps.append(
            {
                "xi": xI,
                "w1pm": w1pm,
                "h_packed": hp,
                "w2i": w2i,
                "bsel3": bsel,
                "b2r3": b2r,
                "e3": e3,
                "maskS": maskS,
            }
        )
    return in_maps


def _unpack_output(res):
    y = np.empty((B, H, W, C), dtype=np.float32)
    for core in range(N_CORES):
        yb = np.asarray(res.results[core]["y_bands"])  # [126, XCOLS] f16
        for s in range(SPC):
            v = yb[:, :].reshape(3, G, XCOLS)[:, G_S * s : G_S * (s + 1), :]
            # v[c, g', n] -> pixel g'*XCOLS + n
            flat = v.transpose(1, 2, 0).reshape(G_S * XCOLS, C)[:PX_SAMPLE]
            y[core * SPC + s] = flat.reshape(H, W, C).astype(np.float32)
    return y


def run(trace=False, **inputs):
    if "nc" not in _CACHE:
        _CACHE["nc"] = _build()
    nc = _CACHE["nc"]
    in_maps = _prep_inputs(**inputs)
    res = bass_utils.run_bass_kernel_spmd(
        nc, in_maps, core_ids=list(range(N_CORES)), trace=trace
    )
    y = _unpack_output(res)
    return y, res


def kernel(**inputs) -> np.ndarray:
    y, _ = run(trace=False, **inputs)
    return y


if __name__ == "__main__":
    rng = np.random.default_rng(0)
    ins = {
        "x": rng.random((B, H, W, C), dtype=np.float32),
        "histogram": rng.random((B, 3, 64, 64), dtype=np.float32),
        "W1": (rng.standard_normal((HIST, HID)) / np.sqrt(HIST)).astype(np.float32),
        "b1": np.zeros(HID, np.float32),
        "W2": (rng.standard_normal((HID, MOUT)) / np.sqrt(HID)).astype(np.float32),
        "b2": np.zeros(MOUT, np.float32),
    }
    y = kernel(**ins)
    print("out", y.shape, y.dtype, float(np.abs(y).max()))
